# revision 11
# baseline (speedup 1.0000x reference)
"""Trainium2 Bass kernel for stacked-Linear dense MLP:
    out[1024, 32768] = x[1024, 512] @ W[32768, 512].T + b[32768]

Strategy: column-parallel over 8 NeuronCores. Core c owns W rows
[c*4096, (c+1)*4096) -> output columns of the same range; x replicated.
On-chip: bf16 matmul (fp32 PSUM accumulate), bias added on DVE during
PSUM->SBUF evacuation (cast to bf16), bf16 output upcast to fp32 on host.

Perf structure:
  - Host pre-arranges x/W into SBUF-image layouts (1-4 KiB contiguous
    per partition per DMA descriptor).
  - n-OUTER loop: each W chunk (512 KiB) feeds 8 m-tile matmul groups
    (~6.8us of PE work vs ~1.5-3us of load), PE runs dense once started.
  - W chunk DMAs are chained (each waits on the previous) so chunk 0
    completes ASAP instead of all chunks time-sharing bandwidth and
    completing together at the end.
  - x DMAs likewise split into 4 chained chunks on the other HWDGE ring.
  - bias DMA + gpsimd partition_broadcast issued first (hidden).
  - PE warmup matmuls un-throttle the HAM clock gate before real work.
  - Output DMAs (bf16, half the bytes of fp32) alternate across rings.
"""

import sys

sys.path.insert(0, "/opt/trn_rl_repo")

import numpy as np
import ml_dtypes

# ---- problem constants (hardcoded per contract) ----
B = 1024          # batch (matmul M)
K = 512           # hidden size (contraction)
N_TOTAL = 32768   # hidden_size * map_element_size
N_CORES = 8
NS = N_TOTAL // N_CORES  # 4096 output cols per core

KT = K // 128     # 4 k-tiles
MT = B // 128     # 8 m-tiles
NCH = NS // 512   # 8 n-chunks of 512 (one PSUM bank each)

OUT_BF16 = True   # device writes bf16, host upcasts to fp32

_CACHE = {}


def _build_program():
    import concourse.bacc as bacc
    import concourse.mybir as mybir
    from concourse.bass import ds, ts
    from concourse.tile import TileContext
    from concourse.tile_rust import add_dep_helper
    from contextlib import ExitStack

    nc = bacc.Bacc("TRN2", target_bir_lowering=False, debug=False)

    out_dt = mybir.dt.bfloat16 if OUT_BF16 else mybir.dt.float32

    # host-prepared SBUF-image layouts (see _prep_inputs)
    xh = nc.dram_tensor("xh", [128, MT, KT, 128], mybir.dt.bfloat16, kind="ExternalInput").ap()
    wh = nc.dram_tensor("wh", [128, NCH, KT, 512], mybir.dt.bfloat16, kind="ExternalInput").ap()
    bias = nc.dram_tensor("bias", [1, NS], mybir.dt.float32, kind="ExternalInput").ap()
    out = nc.dram_tensor("out", [B, NS], out_dt, kind="ExternalOutput").ap()

    with TileContext(nc) as tc:
        with ExitStack() as ctx:
            const = ctx.enter_context(tc.tile_pool(name="const", bufs=1))
            outp = ctx.enter_context(tc.tile_pool(name="outp", bufs=12))
            psum = ctx.enter_context(tc.tile_pool(name="psum", bufs=7, space="PSUM"))
            wpool = ctx.enter_context(tc.tile_pool(name="wpool", bufs=1))

            # --- PE warmup ASAP: gpsimd memset (vector is busy with preamble
            # table loads) + warmup matmuls un-throttle HAM before real work
            warm = const.tile([128, 512], mybir.dt.bfloat16, tag="warm")
            warm_ps = psum.tile([128, 512], mybir.dt.float32, tag="warmps", bufs=1)
            nc.gpsimd.memset(warm[:], 0)
            for _ in range(10):
                nc.tensor.matmul(
                    warm_ps[:], lhsT=warm[:, 0:128], rhs=warm[:], start=True, stop=True
                )
            warm_sink = const.tile([128, 512], mybir.dt.float32, tag="warmsink")
            nc.vector.tensor_copy(warm_sink[:], warm_ps[:])  # keep warmups live

            # --- bias on the sync ring (tiny) + per-chunk gpsimd broadcasts
            bias_sb = const.tile([128, NS], mybir.dt.float32, tag="bias")
            nc.sync.dma_start(bias_sb[0:1, :], bias)
            for n in range(NCH):
                nc.gpsimd.partition_broadcast(
                    bias_sb[:, ds(n * 512, 512)], bias_sb[0:1, ds(n * 512, 512)]
                )

            # --- x on the scalar ring: m0-1 first (critical), rest chained
            xh_sb = const.tile([128, MT, KT, 128], mybir.dt.bfloat16, tag="xh")
            dma_x0 = nc.scalar.dma_start(xh_sb[:, ds(0, 2)], xh[:, ds(0, 2)])
            dma_x1 = nc.scalar.dma_start(xh_sb[:, ds(2, 3)], xh[:, ds(2, 3)])
            dma_x2 = nc.scalar.dma_start(xh_sb[:, ds(5, 3)], xh[:, ds(5, 3)])
            add_dep_helper(dma_x1.ins, dma_x0.ins, reason="chain xh DMAs")
            add_dep_helper(dma_x2.ins, dma_x1.ins, reason="chain xh DMAs")

            # --- W on the sync ring: chained chunks of [1,2,2,3] n-chunks.
            # First link = n0 as two concurrent half DMAs (single-DMA BW is
            # ~100 GB/s; two in flight pipeline better); later links big
            # (amortize per-DMA cost, still far ahead of the PE's
            # 6.8us-per-n-chunk consumption rate).
            W_SPLIT = [1, 2, 2, 3]
            wt_tiles = []
            n2cl = {}
            n0 = 0
            t0 = wpool.tile([128, 1, KT, 512], mybir.dt.bfloat16, tag="wt0")
            dma_w0a = nc.sync.dma_start(t0[:, :, 0:2], wh[:, 0:1, 0:2])
            dma_w0b = nc.sync.dma_start(t0[:, :, 2:4], wh[:, 0:1, 2:4])
            wt_tiles.append(t0)
            n2cl[0] = (0, 0)
            n0 = 1
            prev_list = [dma_w0a, dma_w0b]
            for c, sz in enumerate(W_SPLIT[1:], start=1):
                t = wpool.tile([128, sz, KT, 512], mybir.dt.bfloat16, tag=f"wt{c}")
                dma = nc.sync.dma_start(t[:], wh[:, ds(n0, sz)])
                for p in prev_list:
                    add_dep_helper(dma.ins, p.ins, reason="chain W DMAs")
                prev_list = [dma]
                wt_tiles.append(t)
                for i in range(sz):
                    n2cl[n0 + i] = (c, i)
                n0 += sz

            # --- main loop: n-chunks outer so PE tracks W arrival
            for n in range(NCH):
                for m in range(MT):
                    g = n * MT + m
                    c, ln = n2cl[n]
                    ps = psum.tile([128, 512], mybir.dt.float32)
                    for k in range(KT):
                        nc.tensor.matmul(
                            ps[:],
                            lhsT=xh_sb[:, m, k, :],
                            rhs=wt_tiles[c][:, ln, k, :],
                            start=(k == 0),
                            stop=(k == KT - 1),
                        )
                    ot = outp.tile([128, 512], out_dt)
                    nc.vector.tensor_add(ot[:], ps[:], bias_sb[:, ds(n * 512, 512)])
                    # keep the sync ring clear for the W chain early on
                    if g < 20:
                        eng = nc.scalar
                    else:
                        eng = nc.sync if g % 2 == 0 else nc.scalar
                    eng.dma_start(out[ts(m, 128), ds(n * 512, 512)], ot[:])

    nc.compile()
    return nc


def _get_program():
    if "nc" not in _CACHE:
        _CACHE["nc"] = _build_program()
    return _CACHE["nc"]


def _prep_inputs(x, W, b):
    bf16 = ml_dtypes.bfloat16
    x = np.asarray(x, dtype=np.float32)
    W = np.asarray(W, dtype=np.float32)
    b = np.asarray(b, dtype=np.float32)
    # xh[p, mt, kt, m] = x[mt*128 + m, kt*128 + p]
    xh = np.ascontiguousarray(
        x.T.reshape(KT, 128, MT, 128).transpose(1, 2, 0, 3)
    ).astype(bf16)
    in_maps = []
    for c in range(N_CORES):
        sl = slice(c * NS, (c + 1) * NS)
        # wh[p, n, kt, j] = W[c*NS + n*512 + j, kt*128 + p]
        wh = np.ascontiguousarray(
            W[sl, :].T.reshape(KT, 128, NCH, 512).transpose(1, 2, 0, 3)
        ).astype(bf16)
        bc = np.ascontiguousarray(b[sl].reshape(1, NS))
        in_maps.append({"xh": xh, "wh": wh, "bias": bc})
    return in_maps


def _run(x, W, b, trace=False):
    from concourse.bass_utils import run_bass_kernel_spmd

    nc = _get_program()
    in_maps = _prep_inputs(x, W, b)
    res = run_bass_kernel_spmd(nc, in_maps, list(range(N_CORES)), trace=trace)
    _CACHE["last_result"] = res
    out = np.concatenate([r["out"] for r in res.results], axis=1)
    return out.astype(np.float32)


def kernel(x, W, b):
    return _run(x, W, b, trace=False)


def kernel_profiled(x, W, b):
    """Same as kernel() but with NTFF tracing; returns (out, BassKernelResults)."""
    out = _run(x, W, b, trace=True)
    return out, _CACHE["last_result"]
